# revision 7
# baseline (speedup 1.0000x reference)
"""Trainium2 Bass kernel for nn_ComplexMixture -- raw bacc (no TileContext).

Same algorithm as kernel.py (three triangular Grams M1=A^T A, M2=B^T B,
M3=(A-B)^T(A+B); host combines out_r=M1+M2, out_i=M1-M2-M3), but
hand-scheduled with 8 explicit semaphores instead of Tile's lazy
allocator.  Tile's exit path clears+checks a ~254-semaphore range through
an all-engine "EVSEM butterfly" that costs ~8.5us of measured time; the
raw version replaces it with receipt waits + one all-engine barrier.

Synchronization graph (engines are FIFO queues; sems only cross engines):
  sa/sb   : input DMA receipts (HWDGE per-ring FIFO => monotonic counts)
  szw     : prewarm-weights memset -> PE
  sp      : S/D prep tensor_tensors -> wave-3 matmuls
  smm     : each PSUM bank's final (stop) matmul -> its evacuation
  sevs/sevv: ScalarE / VectorE evacuations -> next wave's matmuls on the
            same bank (WAR) and sync-queue store doorbells
  ssts/ssty: store receipts -> final barrier

PSUM: row-tile accumulators [128, 768-128m] f32 = 2+2+1+1+1+1 = exactly
8 banks, allocated once and reused by all three waves (guarded by
smm/sev chains).  The prewarm writes bank 0 before wave 1 reclaims it
(same-engine program order; start=True clears).
"""

import contextlib
import sys
import types

import numpy as np

try:
    import antenv.axon_hooks  # noqa: F401
except ImportError:
    _hooks = types.ModuleType("antenv.axon_hooks")
    _hooks._hook = None
    _hooks.set_axon_ntff_profile_hook = lambda h: setattr(_hooks, "_hook", h)
    _hooks.get_axon_ntff_profile_hook = lambda: _hooks._hook
    sys.modules["antenv.axon_hooks"] = _hooks

import concourse.bacc as bacc
import concourse.bass_utils as bass_utils
import concourse.mybir as mybir

B, S, D = 8, 512, 768
P = 128
KC = S // P      # 4 contraction chunks
MT = D // P      # 6 row tiles
N_CORES = 8
N_PREWARM = 8    # ~3.4us of cold N=512 matmuls bridging the preamble to
                 # the first input chunk; HAM releases >=3.4us after PE start

WIDTHS = [D - P * m for m in range(MT)]          # 768,640,512,384,256,128
ORDER = [0, 2, 4, 1, 3, 5]                       # scalar block | sync block
OFFS = [0] * MT
_off = 0
for _m in ORDER:
    OFFS[_m] = _off
    _off += WIDTHS[_m]
TRI = _off                                       # 2688

_CACHE: dict = {}


def _build():
    f16, f32 = mybir.dt.float16, mybir.dt.float32
    nc = bacc.Bacc(
        "TRN2", target_bir_lowering=False, debug=False, num_devices=N_CORES
    )
    a_d = nc.dram_tensor("a_in", [P, KC * D], f16, kind="ExternalInput").ap()
    b_d = nc.dram_tensor("b_in", [P, KC * D], f16, kind="ExternalInput").ap()
    m_d = [
        nc.dram_tensor(f"m{x}_out", [P, TRI], f16, kind="ExternalOutput").ap()
        for x in (1, 2, 3)
    ]

    with contextlib.ExitStack() as ctx:
        sb_t = lambda n, sh, dt: ctx.enter_context(nc.sbuf_tensor(n, sh, dt))
        zw = sb_t("zw", [P, 5 * P], f16)
        a01 = sb_t("a01", [P, 2 * D], f16)
        a2 = sb_t("a2", [P, D], f16)
        a3 = sb_t("a3", [P, D], f16)
        b01 = sb_t("b01", [P, 2 * D], f16)
        b23 = sb_t("b23", [P, 2 * D], f16)
        st = [sb_t(f"s{k}", [P, D], f16) for k in range(KC)]
        dt_ = [sb_t(f"d{k}", [P, D], f16) for k in range(KC)]
        ev = [
            [sb_t(f"ev{w}_{m}", [P, WIDTHS[m]], f16) for m in range(MT)]
            for w in range(3)
        ]
        ps = [
            ctx.enter_context(nc.psum_tensor(f"ps{m}", [P, WIDTHS[m]], f32))
            for m in range(MT)
        ]
        sem = lambda n: ctx.enter_context(nc.semaphore(n))
        sa01, sa2, sa3 = sem("sa01"), sem("sa2"), sem("sa3")
        sb01, sb23 = sem("sb01"), sem("sb23")
        szw, sp, smm = sem("szw"), sem("sp"), sem("smm")
        sevs, sevv = sem("sevs"), sem("sevv")
        ssts, ssty = sem("ssts"), sem("ssty")

        def ak(k):
            return (a01[:, 0:D], a01[:, D : 2 * D], a2[:], a3[:])[k]

        def bk(k):
            return (b01[:, 0:D], b01[:, D : 2 * D],
                    b23[:, 0:D], b23[:, D : 2 * D])[k]

        # --- input doorbells.  A rides the sync HWDGE ring as
        # k01+k2+k3 (the scalar queue is blocked ~1.3us by its
        # ACT_TABLE_LOAD, so sync's ring starts first); B rides the
        # scalar ring as two DMAs.  Ring FIFO preserves intra-ring
        # order; each chunk has its own receipt semaphore. ---
        nc.sync.dma_start(a01[:], a_d[:, 0 : 2 * D]).then_inc(sa01, 16)
        nc.scalar.dma_start(b01[:], b_d[:, 0 : 2 * D]).then_inc(sb01, 16)
        nc.sync.dma_start(a2[:], a_d[:, 2 * D : 3 * D]).then_inc(sa2, 16)
        nc.sync.dma_start(a3[:], a_d[:, 3 * D : 4 * D]).then_inc(sa3, 16)
        nc.scalar.dma_start(b23[:], b_d[:, 2 * D : 4 * D]).then_inc(sb23, 16)

        # --- prewarm: PE busy through one full HAM window; memset on
        # GpSimd, whose queue clears the framework preamble earliest ---
        nc.gpsimd.memset(zw[:], 0.0).then_inc(szw)
        nc.tensor.wait_ge(szw, 1)
        for _ in range(N_PREWARM):
            nc.tensor.matmul(
                ps[0][:, 0:512], zw[:, 0:P], zw[:, P : 5 * P],
                start=True, stop=True,
            )

        # --- S/D prep for k0/k1 on VectorE (k2/k3 prep is emitted after
        # the wave-1 evacuations so those evacs unblock wave 2 promptly;
        # everything is consumed only by wave 3) ---
        nc.vector.wait_ge(sa01, 16)
        nc.vector.wait_ge(sb01, 16)
        for k in (0, 1):
            nc.vector.tensor_add(st[k][:], ak(k), bk(k))
            nc.vector.tensor_sub(dt_[k][:], ak(k), bk(k))

        def mm_unit(ps_t, lf, rf, m, k, stt, spp, inc=False):
            lhsT = lf(k)[:, P * m : P * (m + 1)]
            n = WIDTHS[m]
            regs = list(range(0, n, 512))
            for i, c0 in enumerate(regs):
                c1 = min(c0 + 512, n)
                inst = nc.tensor.matmul(
                    ps_t[:, c0:c1], lhsT,
                    rf(k)[:, P * m + c0 : P * m + c1],
                    start=stt, stop=spp,
                )
                if inc and i == len(regs) - 1:
                    inst.then_inc(smm)

        smm_cnt = [0]
        nevs, nevv, nsts, nsty = [0], [0], [0], [0]

        def evac_store(w, m):
            smm_cnt[0] += 1
            dst = m_d[w][:, OFFS[m] : OFFS[m] + WIDTHS[m]]
            e = ev[w][m]
            if m % 2 == 0:
                nc.scalar.wait_ge(smm, smm_cnt[0])
                nc.scalar.copy(e[:], ps[m][:]).then_inc(sevs)
                nevs[0] += 1
                nc.scalar.wait_ge(sevs, nevs[0])
                nc.scalar.dma_start(dst, e[:]).then_inc(ssts, 16)
                nsts[0] += 16
            else:
                nc.vector.wait_ge(smm, smm_cnt[0])
                nc.vector.tensor_copy(e[:], ps[m][:]).then_inc(sevv)
                nevv[0] += 1
                nc.sync.wait_ge(sevv, nevv[0])
                nc.sync.dma_start(dst, e[:]).then_inc(ssty, 16)
                nsty[0] += 16

        def war_wait(w, m):
            # wave w's matmuls on bank m must wait for wave w-1's evac
            if w == 0:
                return
            if m % 2 == 0:
                nc.tensor.wait_ge(sevs, 3 * (w - 1) + m // 2 + 1)
            else:
                nc.tensor.wait_ge(sevv, 3 * (w - 1) + m // 2 + 1)

        # --- wave 1: M1 = A^T A, k-outer (consume a-chunks as they land);
        # smm increments happen in m order during the k=3 round ---
        nc.tensor.wait_ge(sa01, 16)
        for k in range(KC):
            if k == 2:
                nc.tensor.wait_ge(sa2, 16)
            if k == 3:
                nc.tensor.wait_ge(sa3, 16)
            for m in range(MT):
                mm_unit(ps[m], ak, ak, m, k, k == 0, k == KC - 1,
                        inc=(k == KC - 1))
        for m in range(MT):
            evac_store(0, m)

        # late prep: S/D for k2/k3 (after wave-1 evac emission so the
        # vector queue runs those evacs first)
        nc.vector.wait_ge(sa2, 16)
        nc.vector.wait_ge(sb23, 16)
        nc.vector.tensor_add(st[2][:], ak(2), bk(2))
        nc.vector.tensor_sub(dt_[2][:], ak(2), bk(2))
        nc.vector.wait_ge(sa3, 16)
        nc.vector.tensor_add(st[3][:], ak(3), bk(3))
        nc.vector.tensor_sub(dt_[3][:], ak(3), bk(3)).then_inc(sp)

        # --- wave 2: M2 = B^T B, k-inner per row tile ---
        nc.tensor.wait_ge(sb01, 16)
        nc.tensor.wait_ge(sb23, 16)
        for m in range(MT):
            war_wait(1, m)
            for k in range(KC):
                mm_unit(ps[m], bk, bk, m, k, k == 0, k == KC - 1,
                        inc=(k == KC - 1))
            evac_store(1, m)

        # --- wave 3: M3 = Dd^T Ss, k-inner per row tile ---
        nc.tensor.wait_ge(sp, 1)
        lf = lambda k: dt_[k][:]
        rf = lambda k: st[k][:]
        for m in range(MT):
            war_wait(2, m)
            for k in range(KC):
                mm_unit(ps[m], lf, rf, m, k, k == 0, k == KC - 1,
                        inc=(k == KC - 1))
            evac_store(2, m)

        # --- completion: the NEFF validator requires sem-synchronized
        # DMAs, so the stores carry receipt increments and the issuing
        # queues wait for them at the end. ---
        nc.scalar.wait_ge(ssts, nsts[0])
        nc.sync.wait_ge(ssty, nsty[0])

    nc.compile()
    return nc


def get_nc():
    if "nc" not in _CACHE:
        _CACHE["nc"] = _build()
    return _CACHE["nc"]


def make_in_maps(input_real, input_imag, weight):
    input_real = np.asarray(input_real)
    input_imag = np.asarray(input_imag)
    weight = np.asarray(weight, dtype=np.float32)
    sq = np.sqrt(weight)[:, :, None]  # [B, S, 1]
    a16 = (
        (input_real * sq).astype(np.float16)
        .reshape(B, KC, P, D).transpose(0, 2, 1, 3).reshape(B, P, KC * D)
    )
    b16 = (
        (input_imag * sq).astype(np.float16)
        .reshape(B, KC, P, D).transpose(0, 2, 1, 3).reshape(B, P, KC * D)
    )
    return [
        {
            "a_in": np.ascontiguousarray(a16[b]),
            "b_in": np.ascontiguousarray(b16[b]),
        }
        for b in range(B)
    ]


def _unpack_tri(c):
    m = np.zeros((D, D), dtype=np.float32)
    for t in range(MT):
        m[P * t : P * (t + 1), P * t :] = c[:, OFFS[t] : OFFS[t] + WIDTHS[t]]
    return m


def combine(m1c, m2c, m3c):
    m1 = _unpack_tri(np.asarray(m1c, dtype=np.float32))
    m2 = _unpack_tri(np.asarray(m2c, dtype=np.float32))
    m3 = _unpack_tri(np.asarray(m3c, dtype=np.float32))
    out_r = m1 + m2
    out_i = m1 - m2 - m3
    iu = np.triu_indices(D, 1)
    il = (iu[1], iu[0])
    out_r[il] = out_r[iu]
    out_i[il] = -out_i[iu]
    np.fill_diagonal(out_i, 0.0)
    return out_r, out_i


def run(input_real, input_imag, weight, **spmd_kwargs):
    nc = get_nc()
    res = bass_utils.run_bass_kernel_spmd(
        nc,
        make_in_maps(input_real, input_imag, weight),
        core_ids=list(range(N_CORES)),
        **spmd_kwargs,
    )
    out_r = np.empty((B, D, D), dtype=np.float32)
    out_i = np.empty((B, D, D), dtype=np.float32)
    for b in range(B):
        r = res.results[b]
        out_r[b], out_i[b] = combine(r["m1_out"], r["m2_out"], r["m3_out"])
    return (out_r, out_i), res


def kernel(input_real, input_imag, weight):
    (out_r, out_i), _ = run(input_real, input_imag, weight)
    return (out_r, out_i)
